# revision 29
# baseline (speedup 1.0000x reference)
"""MANN (phase-blended mixture-of-experts) forward pass on 8 Trainium2 cores.

Strategy (data-parallel, per sharding hint):
  - Shard batch B=512 across 8 cores (64 samples each); replicate all weights.
  - Weights are cast to bf16 on host and loaded ONCE into SBUF (11.7 MB,
    ~90 KB/partition) where they stay resident; the steady-state body does
    no weight DMA at all, so the kernel runs at the PE roofline instead of
    the HBM roofline.  (fp32 streaming mode kept for reference: DMA-bound
    at ~385 GB/s/core = ~60 us/body.)
  - Host-side prep: transpose expert weights to [K, IN, OUT], pad layer-1
    input dim 480 -> 512, pre-gather the gating columns.
  - Device: activations kept transposed [feat, B].  Key algebraic trick:
        y = sum_k g[:,k] * (x @ Wk[k].T)  ==  sum_k ((g[:,k]*x) @ Wk[k].T)
    so scaling the stationary activations by g[:,k] lets all 8 experts x 4
    K-subtiles accumulate into a single PSUM tile per layer.  The blended
    bias g @ bk is one extra small matmul into the same PSUM group.
  - Pair mode: even/odd experts run concurrently in disjoint 64-col groups
    of the PE array (tile_position=(0,64)), summed on DVE after.
  - ELU built from primitives: elu(x) = max(x, min(exp(x),1) - 1).

Env knobs: MANN_MM_MODE (bf16|fp32), MANN_RESIDENT (1|0), MANN_PAIR (1|0),
MANN_BENCH_REPEAT (body unroll count for slope timing).
"""

import json
import os

import numpy as np
import ml_dtypes

import concourse.bass as bass
import concourse.bass2jax as bass2jax
import concourse.mybir as mybir
import concourse.tile as tile
from concourse import bass_utils as _bass_utils
from concourse.bass_utils import run_bass_kernel_spmd
from concourse.masks import make_identity


def _legalize_bir(bir_bytes):
    """This container's walrus build rejects instructions carrying more than
    one semaphore wait (setupSyncWait: "Too many sync wait commands" -- hit by
    the Tile kernel-tail Drain).  Equivalent legal form: hoist all but one
    wait onto single-wait NoOps immediately preceding the instruction on the
    same engine (sequencers process waits in program order)."""
    data = json.loads(bir_bytes)
    n = 0
    for fn in data.get("functions", []):
        for bb in fn.get("blocks", []):
            out = []
            for inst in bb.get("instructions", []):
                si = inst.get("sync_info")
                waits = si.get("on_wait", []) if si else []
                if len(waits) > 1:
                    for w in waits[:-1]:
                        n += 1
                        out.append({
                            "debug": inst.get("debug", 0),
                            "engine": inst["engine"],
                            "ins": [], "outs": [],
                            "name": f"I-mwfix-{n}",
                            "opcode": "NoOp",
                            "sync_info": {"on_update": [], "on_wait": [w]},
                        })
                    si["on_wait"] = [waits[-1]]
                out.append(inst)
            bb["instructions"] = out
    return json.dumps(data).encode()


_orig_compile_bir_kernel = _bass_utils.compile_bir_kernel


def _patched_compile_bir_kernel(bir_json, tmpdir, neff_name="file.neff"):
    return _orig_compile_bir_kernel(_legalize_bir(bir_json), tmpdir,
                                    neff_name=neff_name)


bass2jax.compile_bir_kernel = _patched_compile_bir_kernel
_bass_utils.compile_bir_kernel = _patched_compile_bir_kernel

B, IN_DIM, OUT_DIM, HID, K, GH, NG = 512, 480, 400, 512, 8, 128, 32
N_CORES = 8
BS = B // N_CORES  # 64 samples per core
IN_PAD = 512       # layer-1 contraction dim padded to 4x128
KSUB = 4           # 512 / 128 contraction subtiles (all layers, post-pad)
OUTS = (HID, HID, OUT_DIM)
P = 128

MM_MODE = os.environ.get("MANN_MM_MODE", "bf16")
RESIDENT = os.environ.get("MANN_RESIDENT", "1") == "1"

# Set to the BassKernelResults of the last run (for test harnesses).
LAST_RESULTS = None

_NC_CACHE = {}


def _elu_from(nc, pool, src_ap, out_shape, tag):
    """elu(src) = max(src, min(exp(src), 1) - 1); src may be PSUM or SBUF.
    3 ops, exp directly from src (activations here are small enough that
    exp cannot overflow fp32).  Returns a new SBUF fp32 tile."""
    f32 = mybir.dt.float32
    texp = pool.tile(out_shape, f32, tag=f"{tag}_exp")
    nc.scalar.activation(texp, src_ap, mybir.ActivationFunctionType.Exp)
    nc.vector.tensor_scalar(texp, texp, 1.0, -1.0, mybir.AluOpType.min,
                            mybir.AluOpType.add)
    y = pool.tile(out_shape, f32, tag=f"{tag}_y")
    nc.vector.tensor_tensor(y, src_ap, texp, mybir.AluOpType.max)
    return y


def _build(mode, repeat=1, loop=0):
    f32 = mybir.dt.float32
    if mode == "bf16":
        wdt = mybir.dt.bfloat16
        mmdt = mybir.dt.bfloat16
    else:
        wdt = f32
        mmdt = f32

    nc = bass.Bass()

    xT_d = nc.dram_tensor("xT", [IN_PAD, BS], wdt, kind="ExternalInput")
    ginT_d = nc.dram_tensor("ginT", [NG, BS], f32, kind="ExternalInput")
    w_d = [
        nc.dram_tensor(f"w{l}", [K, IN_PAD if l == 0 else HID, OUTS[l]], wdt,
                       kind="ExternalInput")
        for l in range(3)
    ]
    b_d = [
        nc.dram_tensor(f"b{l}", [K, OUTS[l]], wdt, kind="ExternalInput")
        for l in range(3)
    ]
    gw1_d = nc.dram_tensor("gw1", [NG, GH], f32, kind="ExternalInput")
    gw2_d = nc.dram_tensor("gw2", [GH, GH], f32, kind="ExternalInput")
    gw3_d = nc.dram_tensor("gw3", [GH, K], f32, kind="ExternalInput")
    gb1_d = nc.dram_tensor("gb1", [GH, 1], f32, kind="ExternalInput")
    gb2_d = nc.dram_tensor("gb2", [GH, 1], f32, kind="ExternalInput")
    gb3_d = nc.dram_tensor("gb3", [K, 1], f32, kind="ExternalInput")
    out_d = nc.dram_tensor("out", [BS, OUT_DIM], f32, kind="ExternalOutput")

    w_bufs = int(os.environ.get("MANN_W_BUFS", "24" if mode == "bf16" else "12"))
    with tile.TileContext(nc) as tc:
        with (
            tc.tile_pool(name="consts", bufs=1) as cpool,
            tc.tile_pool(name="w", bufs=(1 if RESIDENT else w_bufs)) as wpool,
            tc.tile_pool(name="stat", bufs=3) as spool,
            tc.tile_pool(name="xt", bufs=2) as xpool,
            tc.tile_pool(name="y", bufs=2) as ypool,
            tc.tile_pool(name="psy", bufs=2, space="PSUM") as pspool,
            tc.tile_pool(name="pstr", bufs=2, space="PSUM") as ptpool,
            tc.tile_pool(name="psg", bufs=1, space="PSUM") as pgpool,
            tc.tile_pool(name="dsc", bufs=2, space="DRAM") as dpool,
        ):
            pools = (cpool, wpool, spool, xpool, ypool, pspool, ptpool,
                     pgpool, dpool)

            # ---- constants ----
            xt0 = cpool.tile([P, KSUB, BS], wdt)
            nc.sync.dma_start(xt0, xT_d.rearrange("(ko p) b -> p ko b", p=P))
            gin = cpool.tile([NG, BS], f32)
            nc.sync.dma_start(gin, ginT_d[:])
            gw1 = cpool.tile([NG, GH], f32)
            nc.sync.dma_start(gw1, gw1_d[:])
            gw2 = cpool.tile([GH, GH], f32)
            nc.sync.dma_start(gw2, gw2_d[:])
            gw3 = cpool.tile([GH, K], f32)
            nc.sync.dma_start(gw3, gw3_d[:])
            gb1 = cpool.tile([GH, 1], f32)
            nc.sync.dma_start(gb1, gb1_d[:])
            gb2 = cpool.tile([GH, 1], f32)
            nc.sync.dma_start(gb2, gb2_d[:])
            gb3 = cpool.tile([K, 1], f32)
            nc.sync.dma_start(gb3, gb3_d[:])
            bts = []
            for l in range(3):
                bt = cpool.tile([K, OUTS[l]], wdt, tag=f"b{l}")
                nc.sync.dma_start(bt, b_d[l][:])
                bts.append(bt)
            ident = cpool.tile([BS, BS], f32)
            make_identity(nc, ident)
            if mode == "bf16":
                ident_h = cpool.tile([BS, BS], mmdt, tag="identh")
                nc.vector.tensor_copy(ident_h, ident)
            else:
                ident_h = ident

            wsl = None
            if RESIDENT:
                # expert weights loaded once, SBUF-resident for all bodies
                wsl = []
                for l in range(3):
                    row = []
                    for e in range(K):
                        t = cpool.tile([P, KSUB, OUTS[l]], wdt,
                                       tag=f"w{l}_{e}")
                        nc.sync.dma_start(
                            t[:, :, : OUTS[l]],
                            w_d[l][e].rearrange("(ko p) n -> p ko n", p=P),
                        )
                        row.append(t)
                    wsl.append(row)

            consts = (xt0, gin, gw1, gw2, gw3, gb1, gb2, gb3, bts,
                      ident_h, wsl)

            if repeat == 0:
                # no-op baseline for dispatch-overhead measurement
                yo = ypool.tile([BS, OUT_DIM], f32, tag="yo")
                nc.vector.memset(yo, 0.0)
                nc.sync.dma_start(out_d[:], yo)
            # Software pipelining across bodies: body n+1's gating stages
            # are emitted between body n's motion layers, so the serial
            # gating chain (PE->ACT->DVE roundtrips) overlaps the previous
            # body's matmul streams instead of stalling the in-order PE
            # queue between bodies.
            gstages, gst = _gating_stages(nc, mode, mmdt, pools, consts)
            for s in gstages:
                s()
            for _rep in range(repeat):
                if _rep + 1 < repeat:
                    nstages, ngst = _gating_stages(nc, mode, mmdt, pools,
                                                   consts)
                else:
                    nstages, ngst = None, None
                _emit_motion(nc, mode, mmdt, wdt, pools, out_d, consts,
                             gst, next_stages=nstages, next_gst=ngst,
                             accum=(_rep > 0))
                gst = ngst

    return nc


def _gating_stages(nc, mode, mmdt, pools, consts):
    """Emit the gating MLP as three stage-callables so the caller can
    interleave them with the previous body's motion layers (software
    pipelining).  Results land in the returned dict: gT_mm, gTb."""
    f32 = mybir.dt.float32
    (cpool, wpool, spool, xpool, ypool, pspool, ptpool, pgpool,
     dpool) = pools
    (xt0, gin, gw1, gw2, gw3, gb1, gb2, gb3, bts, ident_h,
     wsl) = consts
    st = {}

    def stage0():
        # z and exp(z) both read the matmul PSUM directly with the bias
        # folded into the activation, removing one serial step per layer.
        pg1 = pgpool.tile([GH, BS], f32, tag="psg")
        nc.tensor.matmul(pg1, lhsT=gw1, rhs=gin, start=True, stop=True)
        zg1 = ypool.tile([GH, BS], f32, tag="zg1")
        nc.scalar.activation(zg1, pg1,
                             mybir.ActivationFunctionType.Identity,
                             bias=gb1)
        tx1 = ypool.tile([GH, BS], f32, tag="g1_exp")
        nc.scalar.activation(tx1, pg1, mybir.ActivationFunctionType.Exp,
                             bias=gb1)
        nc.vector.tensor_scalar(tx1, tx1, 1.0, -1.0, mybir.AluOpType.min,
                                mybir.AluOpType.add)
        h1 = ypool.tile([GH, BS], f32, tag="g1_y")
        nc.vector.tensor_tensor(h1, zg1, tx1, mybir.AluOpType.max)
        st["h1"] = h1

    def stage1():
        pg2 = pgpool.tile([GH, BS], f32, tag="psg")
        nc.tensor.matmul(pg2, lhsT=gw2, rhs=st["h1"], start=True, stop=True)
        zg2 = ypool.tile([GH, BS], f32, tag="zg2")
        nc.scalar.activation(zg2, pg2,
                             mybir.ActivationFunctionType.Identity,
                             bias=gb2)
        tx2 = ypool.tile([GH, BS], f32, tag="g2_exp")
        nc.scalar.activation(tx2, pg2, mybir.ActivationFunctionType.Exp,
                             bias=gb2)
        nc.vector.tensor_scalar(tx2, tx2, 1.0, -1.0, mybir.AluOpType.min,
                                mybir.AluOpType.add)
        h2 = ypool.tile([GH, BS], f32, tag="g2_y")
        nc.vector.tensor_tensor(h2, zg2, tx2, mybir.AluOpType.max)
        st["h2"] = h2

    def stage2():
        pg3 = pgpool.tile([K, BS], f32, tag="psg")
        nc.tensor.matmul(pg3, lhsT=gw3, rhs=st["h2"], start=True, stop=True)
        gT = ypool.tile([K, BS], f32, tag="gT")
        nc.scalar.activation(gT, pg3,
                             mybir.ActivationFunctionType.Identity,
                             bias=gb3)
        if mode == "bf16":
            gT_mm = ypool.tile([K, BS], mmdt, tag="gTmm")
            nc.vector.tensor_copy(gT_mm, gT)
        else:
            gT_mm = gT
        # replicate g across partitions off the PE: collapse gT to one
        # partition via a DRAM hop, then GPSIMD partition_broadcast
        sdt = mmdt if mode == "bf16" else f32
        gd = dpool.tile([K, BS], sdt, tag="gd")
        nc.sync.dma_start(gd, gT_mm)
        gTb = ypool.tile([P, K, BS], sdt, tag="gTb")
        nc.sync.dma_start(
            gTb,
            gd.rearrange("(o k) b -> o k b", o=1).to_broadcast((P, K, BS)))
        st["gT_mm"] = gT_mm
        st["gTb"] = gTb

    return [stage0, stage1, stage2], st


def _emit_xk(nc, spool, sdt, gTb, xt):
    xk = spool.tile([P, K, KSUB, BS], sdt, tag="xk")
    for e in range(K):
        gslab = gTb[:, e:e + 1, :].to_broadcast((P, 2, BS))
        nc.vector.tensor_tensor(xk[:, e, 0:2, :], xt[:, 0:2, :], gslab,
                                mybir.AluOpType.mult)
        nc.vector.tensor_tensor(xk[:, e, 2:4, :], xt[:, 2:4, :], gslab,
                                mybir.AluOpType.mult)
    return xk


def _emit_motion(nc, mode, mmdt, wdt, pools, out_d, consts, gst,
                 next_stages=None, next_gst=None, accum=False):
    f32 = mybir.dt.float32
    (cpool, wpool, spool, xpool, ypool, pspool, ptpool, pgpool,
     dpool) = pools
    (xt0, gin, gw1, gw2, gw3, gb1, gb2, gb3, bts, ident_h,
     wsl) = consts
    gT_mm = gst["gT_mm"]
    gTb = gst["gTb"]
    sdt = mmdt if mode == "bf16" else f32

    # ---- motion layers ----
    # Each layer's output columns are split into two halves so the DVE/ACT
    # post-processing (ELU) and PE transposes of half 0 overlap the PE
    # matmuls of half 1.
    # Gating stages of the NEXT body are emitted one layer early (stage k
    # before layer k's streams end) so the slow gTb DMA/broadcast chain has
    # a full layer of slack before the next body's first matmul needs it.
    if next_stages is not None:
        next_stages[0]()
    xt = xt0
    for l in range(3):
        outl = OUTS[l]
        halves = [(0, 256), (256, outl)]

        # per-expert scaled stationaries: one broadcast mult per (expert,
        # k-half) instead of 32 tiny mults
        if l == 0 and "xk0" in gst:
            xk = gst["xk0"]
        else:
            xk = _emit_xk(nc, spool, sdt, gTb, xt)

        use_pair = os.environ.get("MANN_PAIR", "1") == "1"
        pss = []
        for h, (lo, hi) in enumerate(halves):
            if use_pair:
                # Two experts run concurrently in disjoint 64-col groups of
                # the PE array (even experts -> psum rows 0:64, odd ->
                # 64:128 via tile_position=(0,64)); summed on DVE after.
                # ks-group outer so the first 8 pairs of the layer depend
                # only on the previous layer's half-0 output.
                ps_full = pspool.tile([2 * BS, 256], f32, tag=f"psy{h}",
                                      name=f"psy{l}_{h}")
                psA = ps_full[0:BS, : hi - lo]
                psB = ps_full[BS:2 * BS, : hi - lo]
                nc.tensor.matmul(psA, lhsT=gT_mm,
                                 rhs=bts[l][:, lo:hi],
                                 start=True, stop=False,
                                 skip_group_check=True)
                for ksg in range(2):
                    for e0 in range(0, K, 2):
                        for ks in (2 * ksg, 2 * ksg + 1):
                            last = (ksg == 1 and e0 == K - 2 and ks == KSUB - 1)
                            nc.tensor.matmul(
                                psA,
                                lhsT=xk[:, e0, ks, :],
                                rhs=wsl[l][e0][:, ks, lo:hi],
                                start=False, stop=last,
                                skip_group_check=True,
                            )
                            nc.tensor.matmul(
                                psB,
                                lhsT=xk[:, e0 + 1, ks, :],
                                rhs=wsl[l][e0 + 1][:, ks, lo:hi],
                                start=(ksg == 0 and e0 == 0 and ks == 0),
                                stop=last,
                                tile_position=(0, BS),
                                skip_group_check=True,
                            )
                pss.append((psA, psB))
            else:
                ps_full = pspool.tile([BS, 256], f32, tag=f"psy{h}",
                                      name=f"psy{l}_{h}")
                ps = ps_full[:, : hi - lo]
                nc.tensor.matmul(ps, lhsT=gT_mm,
                                 rhs=bts[l][:, lo:hi],
                                 start=True, stop=False)
                for ksg in range(2):
                    for e in range(K):
                        for ks in (2 * ksg, 2 * ksg + 1):
                            nc.tensor.matmul(
                                ps,
                                lhsT=xk[:, e, ks, :],
                                rhs=wsl[l][e][:, ks, lo:hi],
                                start=False,
                                stop=(ksg == 1 and e == K - 1
                                      and ks == KSUB - 1),
                            )
                pss.append((ps, None))

        # software pipelining: next body's gating stage l+1 queues behind
        # this layer's streams; after the last layer, pre-scale the next
        # body's layer-0 stationaries so its matmuls start without a DVE
        # bubble
        if next_stages is not None:
            if l < 2:
                next_stages[l + 1]()
            else:
                next_gst["xk0"] = _emit_xk(nc, spool, sdt, next_gst["gTb"],
                                           xt0)

        if l < 2:
            ptr = ptpool.tile([P, KSUB, BS], sdt, tag="ptr")
            for h, (lo, hi) in enumerate(halves):
                psA, psB = pss[h]
                if psB is not None:
                    # DVE may read only one PSUM operand per instruction:
                    # stage psB to SBUF on the scalar engine, then add.
                    zb = ypool.tile([BS, hi - lo], f32, tag=f"zb{h}")
                    nc.scalar.copy(zb, psB)
                    z = ypool.tile([BS, hi - lo], f32, tag=f"z{h}")
                    nc.vector.tensor_tensor(z, psA, zb,
                                            mybir.AluOpType.add)
                    src = z
                else:
                    src = psA
                # elu, with the max-output narrowed to the matmul dtype so
                # the PE transpose runs at 1 cycle/row
                texp = ypool.tile([BS, hi - lo], f32, tag=f"ml{h}_exp")
                nc.scalar.activation(texp, src,
                                     mybir.ActivationFunctionType.Exp)
                nc.vector.tensor_scalar(texp, texp, 1.0, -1.0,
                                        mybir.AluOpType.min,
                                        mybir.AluOpType.add)
                y = ypool.tile([BS, hi - lo], sdt, tag=f"ml{h}_y")
                nc.vector.tensor_tensor(y, src, texp, mybir.AluOpType.max)
                for c in range(2):
                    nc.tensor.transpose(ptr[:, 2 * h + c, :],
                                        y[:, c * P:(c + 1) * P], ident_h)
            # next layer's xk scaling reads ptr (PSUM) directly -- DVE's
            # one-PSUM-operand rule is satisfied since gTb is SBUF
            xt = ptr
        else:
            yo = ypool.tile([BS, OUT_DIM], f32, tag="yo")
            for h, (lo, hi) in enumerate(halves):
                psA, psB = pss[h]
                if psB is not None:
                    zb = ypool.tile([BS, hi - lo], f32, tag=f"zb{h}")
                    nc.scalar.copy(zb, psB)
                    nc.vector.tensor_tensor(yo[:, lo:hi], psA, zb,
                                            mybir.AluOpType.add)
                else:
                    nc.vector.tensor_copy(yo[:, lo:hi], psA)
            if accum:
                # benchmark-repeat builds accumulate so no body is dead code
                nc.gpsimd.dma_start(out_d[:], yo,
                                    accum_op=mybir.AluOpType.add)
            else:
                nc.sync.dma_start(out_d[:], yo)


def _get_nc(mode):
    repeat = int(os.environ.get("MANN_BENCH_REPEAT", "1"))
    loop = int(os.environ.get("MANN_BENCH_LOOP", "0"))
    key = (mode, repeat, loop, RESIDENT)
    if key not in _NC_CACHE:
        _NC_CACHE[key] = _build(mode, repeat, loop)
    return _NC_CACHE[key]


def prepare_inputs(x, gating_idx, GW1, Gb1, GW2, Gb2, GW3, Gb3,
                   Wk1, bk1, Wk2, bk2, Wk3, bk3, mode):
    wnp = ml_dtypes.bfloat16 if mode == "bf16" else np.float32
    f32 = np.float32
    x = np.asarray(x, f32)
    idx = np.asarray(gating_idx).astype(np.int64)

    xT = np.zeros((IN_PAD, B), f32)
    xT[:IN_DIM] = x.T
    ginT = np.ascontiguousarray(x[:, idx].T)

    w1 = np.zeros((K, IN_PAD, HID), f32)
    w1[:, :IN_DIM] = np.asarray(Wk1, f32).transpose(0, 2, 1)
    w2 = np.ascontiguousarray(np.asarray(Wk2, f32).transpose(0, 2, 1))
    w3 = np.ascontiguousarray(np.asarray(Wk3, f32).transpose(0, 2, 1))

    shared = {
        "w0": w1.astype(wnp), "w1": w2.astype(wnp), "w2": w3.astype(wnp),
        "b0": np.asarray(bk1, f32).astype(wnp),
        "b1": np.asarray(bk2, f32).astype(wnp),
        "b2": np.asarray(bk3, f32).astype(wnp),
        "gw1": np.asarray(GW1, f32), "gw2": np.asarray(GW2, f32),
        "gw3": np.asarray(GW3, f32),
        "gb1": np.asarray(Gb1, f32).reshape(GH, 1),
        "gb2": np.asarray(Gb2, f32).reshape(GH, 1),
        "gb3": np.asarray(Gb3, f32).reshape(K, 1),
    }
    in_maps = []
    for c in range(N_CORES):
        m = dict(shared)
        m["xT"] = np.ascontiguousarray(xT[:, c * BS:(c + 1) * BS]).astype(wnp)
        m["ginT"] = np.ascontiguousarray(ginT[:, c * BS:(c + 1) * BS])
        in_maps.append(m)
    return in_maps


def kernel(**inputs):
    global LAST_RESULTS
    mode = MM_MODE
    nc = _get_nc(mode)
    in_maps = prepare_inputs(mode=mode, **inputs)
    trace = os.environ.get("MANN_TRACE", "0") == "1"
    kwargs = {}
    if trace:
        kwargs["trace"] = True
    res = run_bass_kernel_spmd(nc, in_maps, core_ids=list(range(N_CORES)),
                               **kwargs)
    LAST_RESULTS = res
    out = np.concatenate([r["out"] for r in res.results], axis=0)
    return out.astype(np.float32)


# revision 30
# speedup vs baseline: 1.2723x; 1.2723x over previous
"""MANN (phase-blended mixture-of-experts) forward pass on 8 Trainium2 cores.

Steady-state per-body device time: ~11 us (vs 19.3 us baseline, 60.6 us
for the original fp32 streaming kernel).  Bottleneck after these changes is
the PE stream floor itself (~10.7 us: 9.5 us of full-utilization pair-mode
bf16 matmul streams + bias matmuls + gating/transposes).

Strategy (data-parallel, per sharding hint):
  - Shard batch B=512 across 8 cores (64 samples each); replicate all weights.
  - Weights are cast to bf16 on host and loaded ONCE into SBUF (11.7 MB,
    ~90 KB/partition) where they stay resident; the steady-state body does
    no weight DMA at all, so the kernel runs at the PE roofline instead of
    the HBM roofline.  (fp32 streaming mode kept for reference: DMA-bound
    at ~385 GB/s/core = ~60 us/body.)
  - Host-side prep: transpose expert weights to [K, IN, OUT], pad layer-1
    input dim 480 -> 512, pre-gather the gating columns.
  - Device: activations kept transposed [feat, B].  Key algebraic trick:
        y = sum_k g[:,k] * (x @ Wk[k].T)  ==  sum_k ((g[:,k]*x) @ Wk[k].T)
    so scaling the stationary activations by g[:,k] lets all 8 experts x 4
    K-subtiles accumulate into a single PSUM tile per layer.  The blended
    bias g @ bk is one extra small matmul into the same PSUM group.
  - Pair mode: even/odd experts run concurrently in disjoint 64-col groups
    of the PE array (tile_position=(0,64)), summed on DVE after.
  - ELU built from primitives: elu(x) = max(x, min(exp(x),1) - 1).
  - Software pipelining: engines execute their queues in order, so body
    n+1's serial gating chain (PE->ACT->DVE roundtrips x3) would stall the
    PE between bodies.  Gating stage k of body n+1 is emitted before layer
    k of body n (a full layer of slack ahead of each use), and body n+1's
    layer-0 stationary scaling is pre-emitted at the end of body n.
  - g-replication across partitions runs entirely off the PE: gT -> DRAM
    scratch -> one broadcasting DMA back to SBUF (stride-0 partition
    source).  (GPSIMD partition_broadcast and tc.For_i both hit a walrus
    "ISA wrong length" encoder bug in this container.)

Env knobs: MANN_MM_MODE (bf16|fp32), MANN_RESIDENT (1|0), MANN_PAIR (1|0),
MANN_BENCH_REPEAT (body unroll count for slope timing).
"""

import json
import os

import numpy as np
import ml_dtypes

import concourse.bass as bass
import concourse.bass2jax as bass2jax
import concourse.mybir as mybir
import concourse.tile as tile
from concourse import bass_utils as _bass_utils
from concourse.bass_utils import run_bass_kernel_spmd
from concourse.masks import make_identity


def _legalize_bir(bir_bytes):
    """This container's walrus build rejects instructions carrying more than
    one semaphore wait (setupSyncWait: "Too many sync wait commands" -- hit by
    the Tile kernel-tail Drain).  Equivalent legal form: hoist all but one
    wait onto single-wait NoOps immediately preceding the instruction on the
    same engine (sequencers process waits in program order)."""
    data = json.loads(bir_bytes)
    n = 0
    for fn in data.get("functions", []):
        for bb in fn.get("blocks", []):
            out = []
            for inst in bb.get("instructions", []):
                si = inst.get("sync_info")
                waits = si.get("on_wait", []) if si else []
                if len(waits) > 1:
                    for w in waits[:-1]:
                        n += 1
                        out.append({
                            "debug": inst.get("debug", 0),
                            "engine": inst["engine"],
                            "ins": [], "outs": [],
                            "name": f"I-mwfix-{n}",
                            "opcode": "NoOp",
                            "sync_info": {"on_update": [], "on_wait": [w]},
                        })
                    si["on_wait"] = [waits[-1]]
                out.append(inst)
            bb["instructions"] = out
    return json.dumps(data).encode()


_orig_compile_bir_kernel = _bass_utils.compile_bir_kernel


def _patched_compile_bir_kernel(bir_json, tmpdir, neff_name="file.neff"):
    return _orig_compile_bir_kernel(_legalize_bir(bir_json), tmpdir,
                                    neff_name=neff_name)


bass2jax.compile_bir_kernel = _patched_compile_bir_kernel
_bass_utils.compile_bir_kernel = _patched_compile_bir_kernel

B, IN_DIM, OUT_DIM, HID, K, GH, NG = 512, 480, 400, 512, 8, 128, 32
N_CORES = 8
BS = B // N_CORES  # 64 samples per core
IN_PAD = 512       # layer-1 contraction dim padded to 4x128
KSUB = 4           # 512 / 128 contraction subtiles (all layers, post-pad)
OUTS = (HID, HID, OUT_DIM)
P = 128

MM_MODE = os.environ.get("MANN_MM_MODE", "bf16")
RESIDENT = os.environ.get("MANN_RESIDENT", "1") == "1"

# Set to the BassKernelResults of the last run (for test harnesses).
LAST_RESULTS = None

_NC_CACHE = {}


def _elu_from(nc, pool, src_ap, out_shape, tag):
    """elu(src) = max(src, min(exp(src), 1) - 1); src may be PSUM or SBUF.
    3 ops, exp directly from src (activations here are small enough that
    exp cannot overflow fp32).  Returns a new SBUF fp32 tile."""
    f32 = mybir.dt.float32
    texp = pool.tile(out_shape, f32, tag=f"{tag}_exp")
    nc.scalar.activation(texp, src_ap, mybir.ActivationFunctionType.Exp)
    nc.vector.tensor_scalar(texp, texp, 1.0, -1.0, mybir.AluOpType.min,
                            mybir.AluOpType.add)
    y = pool.tile(out_shape, f32, tag=f"{tag}_y")
    nc.vector.tensor_tensor(y, src_ap, texp, mybir.AluOpType.max)
    return y


def _build(mode, repeat=1, loop=0):
    f32 = mybir.dt.float32
    if mode == "bf16":
        wdt = mybir.dt.bfloat16
        mmdt = mybir.dt.bfloat16
    else:
        wdt = f32
        mmdt = f32

    nc = bass.Bass()

    xT_d = nc.dram_tensor("xT", [IN_PAD, BS], wdt, kind="ExternalInput")
    ginT_d = nc.dram_tensor("ginT", [NG, BS], f32, kind="ExternalInput")
    w_d = [
        nc.dram_tensor(f"w{l}", [K, IN_PAD if l == 0 else HID, OUTS[l]], wdt,
                       kind="ExternalInput")
        for l in range(3)
    ]
    b_d = [
        nc.dram_tensor(f"b{l}", [K, OUTS[l]], wdt, kind="ExternalInput")
        for l in range(3)
    ]
    gw1_d = nc.dram_tensor("gw1", [NG, GH], f32, kind="ExternalInput")
    gw2_d = nc.dram_tensor("gw2", [GH, GH], f32, kind="ExternalInput")
    gw3_d = nc.dram_tensor("gw3", [GH, K], f32, kind="ExternalInput")
    gb1_d = nc.dram_tensor("gb1", [GH, 1], f32, kind="ExternalInput")
    gb2_d = nc.dram_tensor("gb2", [GH, 1], f32, kind="ExternalInput")
    gb3_d = nc.dram_tensor("gb3", [K, 1], f32, kind="ExternalInput")
    out_d = nc.dram_tensor("out", [BS, OUT_DIM], f32, kind="ExternalOutput")

    w_bufs = int(os.environ.get("MANN_W_BUFS", "24" if mode == "bf16" else "12"))
    with tile.TileContext(nc) as tc:
        with (
            tc.tile_pool(name="consts", bufs=1) as cpool,
            tc.tile_pool(name="w", bufs=(1 if RESIDENT else w_bufs)) as wpool,
            tc.tile_pool(name="stat", bufs=3) as spool,
            tc.tile_pool(name="xt", bufs=2) as xpool,
            tc.tile_pool(name="y", bufs=2) as ypool,
            tc.tile_pool(name="psy", bufs=2, space="PSUM") as pspool,
            tc.tile_pool(name="pstr", bufs=2, space="PSUM") as ptpool,
            tc.tile_pool(name="psg", bufs=1, space="PSUM") as pgpool,
            tc.tile_pool(name="dsc", bufs=2, space="DRAM") as dpool,
        ):
            pools = (cpool, wpool, spool, xpool, ypool, pspool, ptpool,
                     pgpool, dpool)

            # ---- constants ----
            xt0 = cpool.tile([P, KSUB, BS], wdt)
            nc.sync.dma_start(xt0, xT_d.rearrange("(ko p) b -> p ko b", p=P))
            gin = cpool.tile([NG, BS], f32)
            nc.sync.dma_start(gin, ginT_d[:])
            gw1 = cpool.tile([NG, GH], f32)
            nc.sync.dma_start(gw1, gw1_d[:])
            gw2 = cpool.tile([GH, GH], f32)
            nc.sync.dma_start(gw2, gw2_d[:])
            gw3 = cpool.tile([GH, K], f32)
            nc.sync.dma_start(gw3, gw3_d[:])
            gb1 = cpool.tile([GH, 1], f32)
            nc.sync.dma_start(gb1, gb1_d[:])
            gb2 = cpool.tile([GH, 1], f32)
            nc.sync.dma_start(gb2, gb2_d[:])
            gb3 = cpool.tile([K, 1], f32)
            nc.sync.dma_start(gb3, gb3_d[:])
            bts = []
            for l in range(3):
                bt = cpool.tile([K, OUTS[l]], wdt, tag=f"b{l}")
                nc.sync.dma_start(bt, b_d[l][:])
                bts.append(bt)
            ident = cpool.tile([BS, BS], f32)
            make_identity(nc, ident)
            if mode == "bf16":
                ident_h = cpool.tile([BS, BS], mmdt, tag="identh")
                nc.vector.tensor_copy(ident_h, ident)
            else:
                ident_h = ident

            wsl = None
            if RESIDENT:
                # expert weights loaded once, SBUF-resident for all bodies
                wsl = []
                for l in range(3):
                    row = []
                    for e in range(K):
                        t = cpool.tile([P, KSUB, OUTS[l]], wdt,
                                       tag=f"w{l}_{e}")
                        nc.sync.dma_start(
                            t[:, :, : OUTS[l]],
                            w_d[l][e].rearrange("(ko p) n -> p ko n", p=P),
                        )
                        row.append(t)
                    wsl.append(row)

            consts = (xt0, gin, gw1, gw2, gw3, gb1, gb2, gb3, bts,
                      ident_h, wsl)

            if repeat == 0:
                # no-op baseline for dispatch-overhead measurement
                yo = ypool.tile([BS, OUT_DIM], f32, tag="yo")
                nc.vector.memset(yo, 0.0)
                nc.sync.dma_start(out_d[:], yo)
            # Software pipelining across bodies: body n+1's gating stages
            # are emitted between body n's motion layers, so the serial
            # gating chain (PE->ACT->DVE roundtrips) overlaps the previous
            # body's matmul streams instead of stalling the in-order PE
            # queue between bodies.
            gstages, gst = _gating_stages(nc, mode, mmdt, pools, consts)
            for s in gstages:
                s()
            for _rep in range(repeat):
                if _rep + 1 < repeat:
                    nstages, ngst = _gating_stages(nc, mode, mmdt, pools,
                                                   consts)
                else:
                    nstages, ngst = None, None
                _emit_motion(nc, mode, mmdt, wdt, pools, out_d, consts,
                             gst, next_stages=nstages, next_gst=ngst,
                             accum=(_rep > 0))
                gst = ngst

    return nc


def _gating_stages(nc, mode, mmdt, pools, consts):
    """Emit the gating MLP as three stage-callables so the caller can
    interleave them with the previous body's motion layers (software
    pipelining).  Results land in the returned dict: gT_mm, gTb."""
    f32 = mybir.dt.float32
    (cpool, wpool, spool, xpool, ypool, pspool, ptpool, pgpool,
     dpool) = pools
    (xt0, gin, gw1, gw2, gw3, gb1, gb2, gb3, bts, ident_h,
     wsl) = consts
    st = {}

    def stage0():
        # z and exp(z) both read the matmul PSUM directly with the bias
        # folded into the activation, removing one serial step per layer.
        pg1 = pgpool.tile([GH, BS], f32, tag="psg")
        nc.tensor.matmul(pg1, lhsT=gw1, rhs=gin, start=True, stop=True)
        zg1 = ypool.tile([GH, BS], f32, tag="zg1")
        nc.scalar.activation(zg1, pg1,
                             mybir.ActivationFunctionType.Identity,
                             bias=gb1)
        tx1 = ypool.tile([GH, BS], f32, tag="g1_exp")
        nc.scalar.activation(tx1, pg1, mybir.ActivationFunctionType.Exp,
                             bias=gb1)
        nc.vector.tensor_scalar(tx1, tx1, 1.0, -1.0, mybir.AluOpType.min,
                                mybir.AluOpType.add)
        h1 = ypool.tile([GH, BS], f32, tag="g1_y")
        nc.vector.tensor_tensor(h1, zg1, tx1, mybir.AluOpType.max)
        st["h1"] = h1

    def stage1():
        pg2 = pgpool.tile([GH, BS], f32, tag="psg")
        nc.tensor.matmul(pg2, lhsT=gw2, rhs=st["h1"], start=True, stop=True)
        zg2 = ypool.tile([GH, BS], f32, tag="zg2")
        nc.scalar.activation(zg2, pg2,
                             mybir.ActivationFunctionType.Identity,
                             bias=gb2)
        tx2 = ypool.tile([GH, BS], f32, tag="g2_exp")
        nc.scalar.activation(tx2, pg2, mybir.ActivationFunctionType.Exp,
                             bias=gb2)
        nc.vector.tensor_scalar(tx2, tx2, 1.0, -1.0, mybir.AluOpType.min,
                                mybir.AluOpType.add)
        h2 = ypool.tile([GH, BS], f32, tag="g2_y")
        nc.vector.tensor_tensor(h2, zg2, tx2, mybir.AluOpType.max)
        st["h2"] = h2

    def stage2():
        pg3 = pgpool.tile([K, BS], f32, tag="psg")
        nc.tensor.matmul(pg3, lhsT=gw3, rhs=st["h2"], start=True, stop=True)
        gT = ypool.tile([K, BS], f32, tag="gT")
        nc.scalar.activation(gT, pg3,
                             mybir.ActivationFunctionType.Identity,
                             bias=gb3)
        if mode == "bf16":
            gT_mm = ypool.tile([K, BS], mmdt, tag="gTmm")
            nc.vector.tensor_copy(gT_mm, gT)
        else:
            gT_mm = gT
        # replicate g across partitions off the PE: collapse gT to one
        # partition via a DRAM hop, then GPSIMD partition_broadcast
        sdt = mmdt if mode == "bf16" else f32
        gd = dpool.tile([K, BS], sdt, tag="gd")
        nc.sync.dma_start(gd, gT_mm)
        gTb = ypool.tile([P, K, BS], sdt, tag="gTb")
        nc.sync.dma_start(
            gTb,
            gd.rearrange("(o k) b -> o k b", o=1).to_broadcast((P, K, BS)))
        st["gT_mm"] = gT_mm
        st["gTb"] = gTb

    return [stage0, stage1, stage2], st


def _emit_xk(nc, spool, sdt, gTb, xt):
    xk = spool.tile([P, K, KSUB, BS], sdt, tag="xk")
    for e in range(K):
        gslab = gTb[:, e:e + 1, :].to_broadcast((P, 2, BS))
        nc.vector.tensor_tensor(xk[:, e, 0:2, :], xt[:, 0:2, :], gslab,
                                mybir.AluOpType.mult)
        nc.vector.tensor_tensor(xk[:, e, 2:4, :], xt[:, 2:4, :], gslab,
                                mybir.AluOpType.mult)
    return xk


def _emit_motion(nc, mode, mmdt, wdt, pools, out_d, consts, gst,
                 next_stages=None, next_gst=None, accum=False):
    f32 = mybir.dt.float32
    (cpool, wpool, spool, xpool, ypool, pspool, ptpool, pgpool,
     dpool) = pools
    (xt0, gin, gw1, gw2, gw3, gb1, gb2, gb3, bts, ident_h,
     wsl) = consts
    gT_mm = gst["gT_mm"]
    gTb = gst["gTb"]
    sdt = mmdt if mode == "bf16" else f32

    # ---- motion layers ----
    # Each layer's output columns are split into two halves so the DVE/ACT
    # post-processing (ELU) and PE transposes of half 0 overlap the PE
    # matmuls of half 1.
    # Gating stages of the NEXT body are emitted one layer early (stage k
    # before layer k's streams end) so the slow gTb DMA/broadcast chain has
    # a full layer of slack before the next body's first matmul needs it.
    if next_stages is not None:
        next_stages[0]()
    xt = xt0
    for l in range(3):
        outl = OUTS[l]
        halves = [(0, 256), (256, outl)]

        # per-expert scaled stationaries: one broadcast mult per (expert,
        # k-half) instead of 32 tiny mults
        if l == 0 and "xk0" in gst:
            xk = gst["xk0"]
        else:
            xk = _emit_xk(nc, spool, sdt, gTb, xt)

        use_pair = os.environ.get("MANN_PAIR", "1") == "1"
        pss = []
        for h, (lo, hi) in enumerate(halves):
            if use_pair:
                # Two experts run concurrently in disjoint 64-col groups of
                # the PE array (even experts -> psum rows 0:64, odd ->
                # 64:128 via tile_position=(0,64)); summed on DVE after.
                # ks-group outer so the first 8 pairs of the layer depend
                # only on the previous layer's half-0 output.
                ps_full = pspool.tile([2 * BS, 256], f32, tag=f"psy{h}",
                                      name=f"psy{l}_{h}")
                psA = ps_full[0:BS, : hi - lo]
                psB = ps_full[BS:2 * BS, : hi - lo]
                nc.tensor.matmul(psA, lhsT=gT_mm,
                                 rhs=bts[l][:, lo:hi],
                                 start=True, stop=False,
                                 skip_group_check=True)
                for ksg in range(2):
                    for e0 in range(0, K, 2):
                        for ks in (2 * ksg, 2 * ksg + 1):
                            last = (ksg == 1 and e0 == K - 2 and ks == KSUB - 1)
                            nc.tensor.matmul(
                                psA,
                                lhsT=xk[:, e0, ks, :],
                                rhs=wsl[l][e0][:, ks, lo:hi],
                                start=False, stop=last,
                                skip_group_check=True,
                            )
                            nc.tensor.matmul(
                                psB,
                                lhsT=xk[:, e0 + 1, ks, :],
                                rhs=wsl[l][e0 + 1][:, ks, lo:hi],
                                start=(ksg == 0 and e0 == 0 and ks == 0),
                                stop=last,
                                tile_position=(0, BS),
                                skip_group_check=True,
                            )
                pss.append((psA, psB))
            else:
                ps_full = pspool.tile([BS, 256], f32, tag=f"psy{h}",
                                      name=f"psy{l}_{h}")
                ps = ps_full[:, : hi - lo]
                nc.tensor.matmul(ps, lhsT=gT_mm,
                                 rhs=bts[l][:, lo:hi],
                                 start=True, stop=False)
                for ksg in range(2):
                    for e in range(K):
                        for ks in (2 * ksg, 2 * ksg + 1):
                            nc.tensor.matmul(
                                ps,
                                lhsT=xk[:, e, ks, :],
                                rhs=wsl[l][e][:, ks, lo:hi],
                                start=False,
                                stop=(ksg == 1 and e == K - 1
                                      and ks == KSUB - 1),
                            )
                pss.append((ps, None))

        # software pipelining: next body's gating stage l+1 queues behind
        # this layer's streams; after the last layer, pre-scale the next
        # body's layer-0 stationaries so its matmuls start without a DVE
        # bubble
        if next_stages is not None:
            if l < 2:
                next_stages[l + 1]()
            else:
                next_gst["xk0"] = _emit_xk(nc, spool, sdt, next_gst["gTb"],
                                           xt0)

        if l < 2:
            ptr = ptpool.tile([P, KSUB, BS], sdt, tag="ptr")
            for h, (lo, hi) in enumerate(halves):
                psA, psB = pss[h]
                if psB is not None:
                    # DVE may read only one PSUM operand per instruction:
                    # stage psB to SBUF on the scalar engine, then add.
                    zb = ypool.tile([BS, hi - lo], f32, tag=f"zb{h}")
                    nc.scalar.copy(zb, psB)
                    z = ypool.tile([BS, hi - lo], f32, tag=f"z{h}")
                    nc.vector.tensor_tensor(z, psA, zb,
                                            mybir.AluOpType.add)
                    src = z
                else:
                    src = psA
                # elu, with the max-output narrowed to the matmul dtype so
                # the PE transpose runs at 1 cycle/row
                texp = ypool.tile([BS, hi - lo], f32, tag=f"ml{h}_exp")
                nc.scalar.activation(texp, src,
                                     mybir.ActivationFunctionType.Exp)
                nc.vector.tensor_scalar(texp, texp, 1.0, -1.0,
                                        mybir.AluOpType.min,
                                        mybir.AluOpType.add)
                y = ypool.tile([BS, hi - lo], sdt, tag=f"ml{h}_y")
                nc.vector.tensor_tensor(y, src, texp, mybir.AluOpType.max)
                for c in range(2):
                    nc.tensor.transpose(ptr[:, 2 * h + c, :],
                                        y[:, c * P:(c + 1) * P], ident_h)
            # next layer's xk scaling reads ptr (PSUM) directly -- DVE's
            # one-PSUM-operand rule is satisfied since gTb is SBUF
            xt = ptr
        else:
            yo = ypool.tile([BS, OUT_DIM], f32, tag="yo")
            for h, (lo, hi) in enumerate(halves):
                psA, psB = pss[h]
                if psB is not None:
                    zb = ypool.tile([BS, hi - lo], f32, tag=f"zb{h}")
                    nc.scalar.copy(zb, psB)
                    nc.vector.tensor_tensor(yo[:, lo:hi], psA, zb,
                                            mybir.AluOpType.add)
                else:
                    nc.vector.tensor_copy(yo[:, lo:hi], psA)
            if accum:
                # benchmark-repeat builds accumulate so no body is dead code
                nc.gpsimd.dma_start(out_d[:], yo,
                                    accum_op=mybir.AluOpType.add)
            else:
                nc.sync.dma_start(out_d[:], yo)


def _get_nc(mode):
    repeat = int(os.environ.get("MANN_BENCH_REPEAT", "1"))
    loop = int(os.environ.get("MANN_BENCH_LOOP", "0"))
    key = (mode, repeat, loop, RESIDENT)
    if key not in _NC_CACHE:
        _NC_CACHE[key] = _build(mode, repeat, loop)
    return _NC_CACHE[key]


def prepare_inputs(x, gating_idx, GW1, Gb1, GW2, Gb2, GW3, Gb3,
                   Wk1, bk1, Wk2, bk2, Wk3, bk3, mode):
    wnp = ml_dtypes.bfloat16 if mode == "bf16" else np.float32
    f32 = np.float32
    x = np.asarray(x, f32)
    idx = np.asarray(gating_idx).astype(np.int64)

    xT = np.zeros((IN_PAD, B), f32)
    xT[:IN_DIM] = x.T
    ginT = np.ascontiguousarray(x[:, idx].T)

    w1 = np.zeros((K, IN_PAD, HID), f32)
    w1[:, :IN_DIM] = np.asarray(Wk1, f32).transpose(0, 2, 1)
    w2 = np.ascontiguousarray(np.asarray(Wk2, f32).transpose(0, 2, 1))
    w3 = np.ascontiguousarray(np.asarray(Wk3, f32).transpose(0, 2, 1))

    shared = {
        "w0": w1.astype(wnp), "w1": w2.astype(wnp), "w2": w3.astype(wnp),
        "b0": np.asarray(bk1, f32).astype(wnp),
        "b1": np.asarray(bk2, f32).astype(wnp),
        "b2": np.asarray(bk3, f32).astype(wnp),
        "gw1": np.asarray(GW1, f32), "gw2": np.asarray(GW2, f32),
        "gw3": np.asarray(GW3, f32),
        "gb1": np.asarray(Gb1, f32).reshape(GH, 1),
        "gb2": np.asarray(Gb2, f32).reshape(GH, 1),
        "gb3": np.asarray(Gb3, f32).reshape(K, 1),
    }
    in_maps = []
    for c in range(N_CORES):
        m = dict(shared)
        m["xT"] = np.ascontiguousarray(xT[:, c * BS:(c + 1) * BS]).astype(wnp)
        m["ginT"] = np.ascontiguousarray(ginT[:, c * BS:(c + 1) * BS])
        in_maps.append(m)
    return in_maps


def kernel(**inputs):
    global LAST_RESULTS
    mode = MM_MODE
    nc = _get_nc(mode)
    in_maps = prepare_inputs(mode=mode, **inputs)
    trace = os.environ.get("MANN_TRACE", "0") == "1"
    kwargs = {}
    if trace:
        kwargs["trace"] = True
    res = run_bass_kernel_spmd(nc, in_maps, core_ids=list(range(N_CORES)),
                               **kwargs)
    LAST_RESULTS = res
    out = np.concatenate([r["out"] for r in res.results], axis=0)
    return out.astype(np.float32)


# revision 32
# speedup vs baseline: 6.3465x; 4.9883x over previous
"""MANN (phase-blended mixture-of-experts) forward pass on 8 Trainium2 cores.

Steady-state per-body device time: ~11 us (vs 19.3 us baseline, 60.6 us
for the original fp32 streaming kernel).  Bottleneck after these changes is
the PE stream floor itself (~10.7 us: 9.5 us of full-utilization pair-mode
bf16 matmul streams + bias matmuls + gating/transposes).

Strategy (data-parallel, per sharding hint):
  - Shard batch B=512 across 8 cores (64 samples each); replicate all weights.
  - Weights are cast to bf16 on host and loaded ONCE into SBUF (11.7 MB,
    ~90 KB/partition) where they stay resident; the steady-state body does
    no weight DMA at all, so the kernel runs at the PE roofline instead of
    the HBM roofline.  (fp32 streaming mode kept for reference: DMA-bound
    at ~385 GB/s/core = ~60 us/body.)
  - Host-side prep: transpose expert weights to [K, IN, OUT], pad layer-1
    input dim 480 -> 512, pre-gather the gating columns.
  - Device: activations kept transposed [feat, B].  Key algebraic trick:
        y = sum_k g[:,k] * (x @ Wk[k].T)  ==  sum_k ((g[:,k]*x) @ Wk[k].T)
    so scaling the stationary activations by g[:,k] lets all 8 experts x 4
    K-subtiles accumulate into a single PSUM tile per layer.  The blended
    bias g @ bk is one extra small matmul into the same PSUM group for
    layers 1/2; layer 0's bias rides the zero-padded contract rows (x row
    480 is constant 1.0, weight row 480 holds the bias -- exact for any
    bias, no extra matmul).
  - Pair mode: even/odd experts run concurrently in disjoint 64-col groups
    of the PE array (tile_position=(0,64)), summed on DVE after.
  - ELU built from primitives: elu(x) = max(x, min(exp(x),1) - 1).
  - Software pipelining: engines execute their queues in order, so body
    n+1's serial gating chain (PE->ACT->DVE roundtrips x3) would stall the
    PE between bodies.  Gating stage k of body n+1 is emitted before layer
    k of body n (a full layer of slack ahead of each use), and body n+1's
    layer-0 stationary scaling is pre-emitted at the end of body n.
  - g-replication across partitions runs entirely off the PE: gT -> DRAM
    scratch -> one broadcasting DMA back to SBUF (stride-0 partition
    source).  (GPSIMD partition_broadcast and tc.For_i both hit a walrus
    "ISA wrong length" encoder bug in this container.)

Env knobs: MANN_MM_MODE (bf16|fp32), MANN_RESIDENT (1|0), MANN_PAIR (1|0),
MANN_BENCH_REPEAT (body unroll count for slope timing).
"""

import json
import os

import numpy as np
import ml_dtypes

import concourse.bass as bass
import concourse.bass2jax as bass2jax
import concourse.mybir as mybir
import concourse.tile as tile
from concourse import bass_utils as _bass_utils
from concourse.bass_utils import run_bass_kernel_spmd
from concourse.masks import make_identity


def _legalize_bir(bir_bytes):
    """This container's walrus build rejects instructions carrying more than
    one semaphore wait (setupSyncWait: "Too many sync wait commands" -- hit by
    the Tile kernel-tail Drain).  Equivalent legal form: hoist all but one
    wait onto single-wait NoOps immediately preceding the instruction on the
    same engine (sequencers process waits in program order)."""
    data = json.loads(bir_bytes)
    n = 0
    for fn in data.get("functions", []):
        for bb in fn.get("blocks", []):
            out = []
            for inst in bb.get("instructions", []):
                si = inst.get("sync_info")
                waits = si.get("on_wait", []) if si else []
                if len(waits) > 1:
                    for w in waits[:-1]:
                        n += 1
                        out.append({
                            "debug": inst.get("debug", 0),
                            "engine": inst["engine"],
                            "ins": [], "outs": [],
                            "name": f"I-mwfix-{n}",
                            "opcode": "NoOp",
                            "sync_info": {"on_update": [], "on_wait": [w]},
                        })
                    si["on_wait"] = [waits[-1]]
                out.append(inst)
            bb["instructions"] = out
    return json.dumps(data).encode()


_orig_compile_bir_kernel = _bass_utils.compile_bir_kernel


def _patched_compile_bir_kernel(bir_json, tmpdir, neff_name="file.neff"):
    return _orig_compile_bir_kernel(_legalize_bir(bir_json), tmpdir,
                                    neff_name=neff_name)


bass2jax.compile_bir_kernel = _patched_compile_bir_kernel
_bass_utils.compile_bir_kernel = _patched_compile_bir_kernel

B, IN_DIM, OUT_DIM, HID, K, GH, NG = 512, 480, 400, 512, 8, 128, 32
N_CORES = 8
BS = B // N_CORES  # 64 samples per core
IN_PAD = 512       # layer-1 contraction dim padded to 4x128
KSUB = 4           # 512 / 128 contraction subtiles (all layers, post-pad)
OUTS = (HID, HID, OUT_DIM)
P = 128

MM_MODE = os.environ.get("MANN_MM_MODE", "bf16")
RESIDENT = os.environ.get("MANN_RESIDENT", "1") == "1"

# Set to the BassKernelResults of the last run (for test harnesses).
LAST_RESULTS = None

_NC_CACHE = {}


def _elu_from(nc, pool, src_ap, out_shape, tag):
    """elu(src) = max(src, min(exp(src), 1) - 1); src may be PSUM or SBUF.
    3 ops, exp directly from src (activations here are small enough that
    exp cannot overflow fp32).  Returns a new SBUF fp32 tile."""
    f32 = mybir.dt.float32
    texp = pool.tile(out_shape, f32, tag=f"{tag}_exp")
    nc.scalar.activation(texp, src_ap, mybir.ActivationFunctionType.Exp)
    nc.vector.tensor_scalar(texp, texp, 1.0, -1.0, mybir.AluOpType.min,
                            mybir.AluOpType.add)
    y = pool.tile(out_shape, f32, tag=f"{tag}_y")
    nc.vector.tensor_tensor(y, src_ap, texp, mybir.AluOpType.max)
    return y


def _build(mode, repeat=1, loop=0):
    f32 = mybir.dt.float32
    if mode == "bf16":
        wdt = mybir.dt.bfloat16
        mmdt = mybir.dt.bfloat16
    else:
        wdt = f32
        mmdt = f32

    nc = bass.Bass()

    xT_d = nc.dram_tensor("xT", [IN_PAD, BS], wdt, kind="ExternalInput")
    ginT_d = nc.dram_tensor("ginT", [NG, BS], f32, kind="ExternalInput")
    w_d = [
        nc.dram_tensor(f"w{l}", [K, IN_PAD if l == 0 else HID, OUTS[l]], wdt,
                       kind="ExternalInput")
        for l in range(3)
    ]
    b_d = [
        nc.dram_tensor(f"b{l}", [K, OUTS[l]], wdt, kind="ExternalInput")
        for l in range(3)
    ]
    gw1_d = nc.dram_tensor("gw1", [NG, GH], f32, kind="ExternalInput")
    gw2_d = nc.dram_tensor("gw2", [GH, GH], f32, kind="ExternalInput")
    gw3_d = nc.dram_tensor("gw3", [GH, K], f32, kind="ExternalInput")
    gb1_d = nc.dram_tensor("gb1", [GH, 1], f32, kind="ExternalInput")
    gb2_d = nc.dram_tensor("gb2", [GH, 1], f32, kind="ExternalInput")
    gb3_d = nc.dram_tensor("gb3", [K, 1], f32, kind="ExternalInput")
    out_d = nc.dram_tensor("out", [BS, OUT_DIM], f32, kind="ExternalOutput")

    w_bufs = int(os.environ.get("MANN_W_BUFS", "24" if mode == "bf16" else "12"))
    with tile.TileContext(nc) as tc:
        with (
            tc.tile_pool(name="consts", bufs=1) as cpool,
            tc.tile_pool(name="w", bufs=(1 if RESIDENT else w_bufs)) as wpool,
            tc.tile_pool(name="stat", bufs=3) as spool,
            tc.tile_pool(name="xt", bufs=2) as xpool,
            tc.tile_pool(name="y", bufs=2) as ypool,
            tc.tile_pool(name="psy", bufs=2, space="PSUM") as pspool,
            tc.tile_pool(name="pstr", bufs=2, space="PSUM") as ptpool,
            tc.tile_pool(name="psg", bufs=1, space="PSUM") as pgpool,
            tc.tile_pool(name="dsc", bufs=2, space="DRAM") as dpool,
        ):
            pools = (cpool, wpool, spool, xpool, ypool, pspool, ptpool,
                     pgpool, dpool)

            # ---- constants ----
            xt0 = cpool.tile([P, KSUB, BS], wdt)
            nc.sync.dma_start(xt0, xT_d.rearrange("(ko p) b -> p ko b", p=P))
            gin = cpool.tile([NG, BS], f32)
            nc.sync.dma_start(gin, ginT_d[:])
            gw1 = cpool.tile([NG, GH], f32)
            nc.sync.dma_start(gw1, gw1_d[:])
            gw2 = cpool.tile([GH, GH], f32)
            nc.sync.dma_start(gw2, gw2_d[:])
            gw3 = cpool.tile([GH, K], f32)
            nc.sync.dma_start(gw3, gw3_d[:])
            gb1 = cpool.tile([GH, 1], f32)
            nc.sync.dma_start(gb1, gb1_d[:])
            gb2 = cpool.tile([GH, 1], f32)
            nc.sync.dma_start(gb2, gb2_d[:])
            gb3 = cpool.tile([K, 1], f32)
            nc.sync.dma_start(gb3, gb3_d[:])
            bts = []
            for l in range(3):
                bt = cpool.tile([K, OUTS[l]], wdt, tag=f"b{l}")
                nc.sync.dma_start(bt, b_d[l][:])
                bts.append(bt)
            ident = cpool.tile([BS, BS], f32)
            make_identity(nc, ident)
            if mode == "bf16":
                ident_h = cpool.tile([BS, BS], mmdt, tag="identh")
                nc.vector.tensor_copy(ident_h, ident)
            else:
                ident_h = ident

            wsl = None
            if RESIDENT:
                # expert weights loaded once, SBUF-resident for all bodies
                wsl = []
                for l in range(3):
                    row = []
                    for e in range(K):
                        t = cpool.tile([P, KSUB, OUTS[l]], wdt,
                                       tag=f"w{l}_{e}")
                        nc.sync.dma_start(
                            t[:, :, : OUTS[l]],
                            w_d[l][e].rearrange("(ko p) n -> p ko n", p=P),
                        )
                        row.append(t)
                    wsl.append(row)

            consts = (xt0, gin, gw1, gw2, gw3, gb1, gb2, gb3, bts,
                      ident_h, wsl)

            if repeat == 0:
                # no-op baseline for dispatch-overhead measurement
                yo = ypool.tile([BS, OUT_DIM], f32, tag="yo")
                nc.vector.memset(yo, 0.0)
                nc.sync.dma_start(out_d[:], yo)
            # Software pipelining across bodies: body n+1's gating stages
            # are emitted between body n's motion layers, so the serial
            # gating chain (PE->ACT->DVE roundtrips) overlaps the previous
            # body's matmul streams instead of stalling the in-order PE
            # queue between bodies.
            gstages, gst = _gating_stages(nc, mode, mmdt, pools, consts)
            for s in gstages:
                s()
            for _rep in range(repeat):
                if _rep + 1 < repeat:
                    nstages, ngst = _gating_stages(nc, mode, mmdt, pools,
                                                   consts)
                else:
                    nstages, ngst = None, None
                _emit_motion(nc, mode, mmdt, wdt, pools, out_d, consts,
                             gst, next_stages=nstages, next_gst=ngst,
                             accum=(_rep > 0))
                gst = ngst

    return nc


def _gating_stages(nc, mode, mmdt, pools, consts):
    """Emit the gating MLP as three stage-callables so the caller can
    interleave them with the previous body's motion layers (software
    pipelining).  Results land in the returned dict: gT_mm, gTb."""
    f32 = mybir.dt.float32
    (cpool, wpool, spool, xpool, ypool, pspool, ptpool, pgpool,
     dpool) = pools
    (xt0, gin, gw1, gw2, gw3, gb1, gb2, gb3, bts, ident_h,
     wsl) = consts
    st = {}

    def stage0():
        # z and exp(z) both read the matmul PSUM directly with the bias
        # folded into the activation, removing one serial step per layer.
        pg1 = pgpool.tile([GH, BS], f32, tag="psg")
        nc.tensor.matmul(pg1, lhsT=gw1, rhs=gin, start=True, stop=True)
        zg1 = ypool.tile([GH, BS], f32, tag="zg1")
        nc.scalar.activation(zg1, pg1,
                             mybir.ActivationFunctionType.Identity,
                             bias=gb1)
        tx1 = ypool.tile([GH, BS], f32, tag="g1_exp")
        nc.scalar.activation(tx1, pg1, mybir.ActivationFunctionType.Exp,
                             bias=gb1)
        nc.vector.tensor_scalar(tx1, tx1, 1.0, -1.0, mybir.AluOpType.min,
                                mybir.AluOpType.add)
        h1 = ypool.tile([GH, BS], f32, tag="g1_y")
        nc.vector.tensor_tensor(h1, zg1, tx1, mybir.AluOpType.max)
        st["h1"] = h1

    def stage1():
        pg2 = pgpool.tile([GH, BS], f32, tag="psg")
        nc.tensor.matmul(pg2, lhsT=gw2, rhs=st["h1"], start=True, stop=True)
        zg2 = ypool.tile([GH, BS], f32, tag="zg2")
        nc.scalar.activation(zg2, pg2,
                             mybir.ActivationFunctionType.Identity,
                             bias=gb2)
        tx2 = ypool.tile([GH, BS], f32, tag="g2_exp")
        nc.scalar.activation(tx2, pg2, mybir.ActivationFunctionType.Exp,
                             bias=gb2)
        nc.vector.tensor_scalar(tx2, tx2, 1.0, -1.0, mybir.AluOpType.min,
                                mybir.AluOpType.add)
        h2 = ypool.tile([GH, BS], f32, tag="g2_y")
        nc.vector.tensor_tensor(h2, zg2, tx2, mybir.AluOpType.max)
        st["h2"] = h2

    def stage2():
        pg3 = pgpool.tile([K, BS], f32, tag="psg")
        nc.tensor.matmul(pg3, lhsT=gw3, rhs=st["h2"], start=True, stop=True)
        gT = ypool.tile([K, BS], f32, tag="gT")
        nc.scalar.activation(gT, pg3,
                             mybir.ActivationFunctionType.Identity,
                             bias=gb3)
        if mode == "bf16":
            gT_mm = ypool.tile([K, BS], mmdt, tag="gTmm")
            nc.vector.tensor_copy(gT_mm, gT)
        else:
            gT_mm = gT
        # replicate g across partitions off the PE: collapse gT to one
        # partition via a DRAM hop, then GPSIMD partition_broadcast
        sdt = mmdt if mode == "bf16" else f32
        gd = dpool.tile([K, BS], sdt, tag="gd")
        nc.sync.dma_start(gd, gT_mm)
        gTb = ypool.tile([P, K, BS], sdt, tag="gTb")
        nc.sync.dma_start(
            gTb,
            gd.rearrange("(o k) b -> o k b", o=1).to_broadcast((P, K, BS)))
        st["gT_mm"] = gT_mm
        st["gTb"] = gTb

    return [stage0, stage1, stage2], st


def _emit_xk(nc, spool, sdt, gTb, xt):
    xk = spool.tile([P, K, KSUB, BS], sdt, tag="xk")
    for e in range(K):
        gslab = gTb[:, e:e + 1, :].to_broadcast((P, 2, BS))
        nc.vector.tensor_tensor(xk[:, e, 0:2, :], xt[:, 0:2, :], gslab,
                                mybir.AluOpType.mult)
        nc.vector.tensor_tensor(xk[:, e, 2:4, :], xt[:, 2:4, :], gslab,
                                mybir.AluOpType.mult)
    return xk


def _emit_motion(nc, mode, mmdt, wdt, pools, out_d, consts, gst,
                 next_stages=None, next_gst=None, accum=False):
    f32 = mybir.dt.float32
    (cpool, wpool, spool, xpool, ypool, pspool, ptpool, pgpool,
     dpool) = pools
    (xt0, gin, gw1, gw2, gw3, gb1, gb2, gb3, bts, ident_h,
     wsl) = consts
    gT_mm = gst["gT_mm"]
    gTb = gst["gTb"]
    sdt = mmdt if mode == "bf16" else f32

    # ---- motion layers ----
    # Each layer's output columns are split into two halves so the DVE/ACT
    # post-processing (ELU) and PE transposes of half 0 overlap the PE
    # matmuls of half 1.
    # Gating stages of the NEXT body are emitted one layer early (stage k
    # before layer k's streams end) so the slow gTb DMA/broadcast chain has
    # a full layer of slack before the next body's first matmul needs it.
    if next_stages is not None:
        next_stages[0]()
    xt = xt0
    for l in range(3):
        outl = OUTS[l]
        halves = [(0, 256), (256, outl)]

        # per-expert scaled stationaries: one broadcast mult per (expert,
        # k-half) instead of 32 tiny mults
        if l == 0 and "xk0" in gst:
            xk = gst["xk0"]
        else:
            xk = _emit_xk(nc, spool, sdt, gTb, xt)

        use_pair = os.environ.get("MANN_PAIR", "1") == "1"
        pss = []
        for h, (lo, hi) in enumerate(halves):
            if use_pair:
                # Two experts run concurrently in disjoint 64-col groups of
                # the PE array (even experts -> psum rows 0:64, odd ->
                # 64:128 via tile_position=(0,64)); summed on DVE after.
                # ks-group outer so the first 8 pairs of the layer depend
                # only on the previous layer's half-0 output.
                ps_full = pspool.tile([2 * BS, 256], f32, tag=f"psy{h}",
                                      name=f"psy{l}_{h}")
                psA = ps_full[0:BS, : hi - lo]
                psB = ps_full[BS:2 * BS, : hi - lo]
                if l > 0:
                    nc.tensor.matmul(psA, lhsT=gT_mm,
                                     rhs=bts[l][:, lo:hi],
                                     start=True, stop=False,
                                     skip_group_check=True)
                for ksg in range(2):
                    for e0 in range(0, K, 2):
                        for ks in (2 * ksg, 2 * ksg + 1):
                            last = (ksg == 1 and e0 == K - 2 and ks == KSUB - 1)
                            nc.tensor.matmul(
                                psA,
                                lhsT=xk[:, e0, ks, :],
                                rhs=wsl[l][e0][:, ks, lo:hi],
                                start=(l == 0 and ksg == 0 and e0 == 0
                                       and ks == 0),
                                stop=last,
                                skip_group_check=True,
                            )
                            nc.tensor.matmul(
                                psB,
                                lhsT=xk[:, e0 + 1, ks, :],
                                rhs=wsl[l][e0 + 1][:, ks, lo:hi],
                                start=(ksg == 0 and e0 == 0 and ks == 0),
                                stop=last,
                                tile_position=(0, BS),
                                skip_group_check=True,
                            )
                pss.append((psA, psB))
            else:
                ps_full = pspool.tile([BS, 256], f32, tag=f"psy{h}",
                                      name=f"psy{l}_{h}")
                ps = ps_full[:, : hi - lo]
                if l > 0:
                    nc.tensor.matmul(ps, lhsT=gT_mm,
                                     rhs=bts[l][:, lo:hi],
                                     start=True, stop=False)
                for ksg in range(2):
                    for e in range(K):
                        for ks in (2 * ksg, 2 * ksg + 1):
                            nc.tensor.matmul(
                                ps,
                                lhsT=xk[:, e, ks, :],
                                rhs=wsl[l][e][:, ks, lo:hi],
                                start=(l == 0 and ksg == 0 and e == 0
                                       and ks == 0),
                                stop=(ksg == 1 and e == K - 1
                                      and ks == KSUB - 1),
                            )
                pss.append((ps, None))

        # software pipelining: next body's gating stage l+1 queues behind
        # this layer's streams; after the last layer, pre-scale the next
        # body's layer-0 stationaries so its matmuls start without a DVE
        # bubble
        if next_stages is not None:
            if l < 2:
                next_stages[l + 1]()
            else:
                next_gst["xk0"] = _emit_xk(nc, spool, sdt, next_gst["gTb"],
                                           xt0)

        if l < 2:
            ptr = ptpool.tile([P, KSUB, BS], sdt, tag="ptr")
            for h, (lo, hi) in enumerate(halves):
                psA, psB = pss[h]
                if psB is not None:
                    # DVE may read only one PSUM operand per instruction:
                    # stage psB to SBUF on the scalar engine, then add.
                    zb = ypool.tile([BS, hi - lo], f32, tag=f"zb{h}")
                    nc.scalar.copy(zb, psB)
                    z = ypool.tile([BS, hi - lo], f32, tag=f"z{h}")
                    nc.vector.tensor_tensor(z, psA, zb,
                                            mybir.AluOpType.add)
                    src = z
                else:
                    src = psA
                # elu, with the max-output narrowed to the matmul dtype so
                # the PE transpose runs at 1 cycle/row
                texp = ypool.tile([BS, hi - lo], f32, tag=f"ml{h}_exp")
                nc.scalar.activation(texp, src,
                                     mybir.ActivationFunctionType.Exp)
                nc.vector.tensor_scalar(texp, texp, 1.0, -1.0,
                                        mybir.AluOpType.min,
                                        mybir.AluOpType.add)
                y = ypool.tile([BS, hi - lo], sdt, tag=f"ml{h}_y")
                nc.vector.tensor_tensor(y, src, texp, mybir.AluOpType.max)
                for c in range(2):
                    nc.tensor.transpose(ptr[:, 2 * h + c, :],
                                        y[:, c * P:(c + 1) * P], ident_h)
            # next layer's xk scaling reads ptr (PSUM) directly -- DVE's
            # one-PSUM-operand rule is satisfied since gTb is SBUF
            xt = ptr
        else:
            yo = ypool.tile([BS, OUT_DIM], f32, tag="yo")
            for h, (lo, hi) in enumerate(halves):
                psA, psB = pss[h]
                if psB is not None:
                    zb = ypool.tile([BS, hi - lo], f32, tag=f"zb{h}")
                    nc.scalar.copy(zb, psB)
                    nc.vector.tensor_tensor(yo[:, lo:hi], psA, zb,
                                            mybir.AluOpType.add)
                else:
                    nc.vector.tensor_copy(yo[:, lo:hi], psA)
            if accum:
                # benchmark-repeat builds accumulate so no body is dead code
                nc.gpsimd.dma_start(out_d[:], yo,
                                    accum_op=mybir.AluOpType.add)
            else:
                nc.sync.dma_start(out_d[:], yo)


def _get_nc(mode):
    repeat = int(os.environ.get("MANN_BENCH_REPEAT", "1"))
    loop = int(os.environ.get("MANN_BENCH_LOOP", "0"))
    key = (mode, repeat, loop, RESIDENT)
    if key not in _NC_CACHE:
        _NC_CACHE[key] = _build(mode, repeat, loop)
    return _NC_CACHE[key]


def prepare_inputs(x, gating_idx, GW1, Gb1, GW2, Gb2, GW3, Gb3,
                   Wk1, bk1, Wk2, bk2, Wk3, bk3, mode):
    wnp = ml_dtypes.bfloat16 if mode == "bf16" else np.float32
    f32 = np.float32
    x = np.asarray(x, f32)
    idx = np.asarray(gating_idx).astype(np.int64)

    xT = np.zeros((IN_PAD, B), f32)
    xT[:IN_DIM] = x.T
    # constant-1 row at pad position IN_DIM: together with the layer-0 bias
    # stored as weight row IN_DIM, the blended bias g @ bk1 rides the
    # existing contraction (exact for any bias), dropping layer-0's
    # dedicated bias matmuls
    xT[IN_DIM] = 1.0
    ginT = np.ascontiguousarray(x[:, idx].T)

    w1 = np.zeros((K, IN_PAD, HID), f32)
    w1[:, :IN_DIM] = np.asarray(Wk1, f32).transpose(0, 2, 1)
    w1[:, IN_DIM] = np.asarray(bk1, f32)
    w2 = np.ascontiguousarray(np.asarray(Wk2, f32).transpose(0, 2, 1))
    w3 = np.ascontiguousarray(np.asarray(Wk3, f32).transpose(0, 2, 1))

    shared = {
        "w0": w1.astype(wnp), "w1": w2.astype(wnp), "w2": w3.astype(wnp),
        "b0": np.asarray(bk1, f32).astype(wnp),
        "b1": np.asarray(bk2, f32).astype(wnp),
        "b2": np.asarray(bk3, f32).astype(wnp),
        "gw1": np.asarray(GW1, f32), "gw2": np.asarray(GW2, f32),
        "gw3": np.asarray(GW3, f32),
        "gb1": np.asarray(Gb1, f32).reshape(GH, 1),
        "gb2": np.asarray(Gb2, f32).reshape(GH, 1),
        "gb3": np.asarray(Gb3, f32).reshape(K, 1),
    }
    in_maps = []
    for c in range(N_CORES):
        m = dict(shared)
        m["xT"] = np.ascontiguousarray(xT[:, c * BS:(c + 1) * BS]).astype(wnp)
        m["ginT"] = np.ascontiguousarray(ginT[:, c * BS:(c + 1) * BS])
        in_maps.append(m)
    return in_maps


def kernel(**inputs):
    global LAST_RESULTS
    mode = MM_MODE
    nc = _get_nc(mode)
    in_maps = prepare_inputs(mode=mode, **inputs)
    trace = os.environ.get("MANN_TRACE", "0") == "1"
    kwargs = {}
    if trace:
        kwargs["trace"] = True
    res = run_bass_kernel_spmd(nc, in_maps, core_ids=list(range(N_CORES)),
                               **kwargs)
    LAST_RESULTS = res
    out = np.concatenate([r["out"] for r in res.results], axis=0)
    return out.astype(np.float32)
